# revision 25
# baseline (speedup 1.0000x reference)
"""ContinuousMask kernel for Trainium2 (8 NeuronCores, SPMD row-sharded).

Problem: starts[B=2048, N=8192] int32, T=16384, l=1638. Output bool [B, T]:
True everywhere except the union of windows [s, s+l) over each row's starts.

Algorithm (per row):
  A position t is covered iff some start lies in (t-l, t]. With value-chunks
  of width W=512 (2W <= l), if every chunk 0..(smax>>9)-1 contains at least
  one start, then the covered region is EXACTLY [smin, smax+l):
    - t in [smin, smin+l): covered by the smin window.
    - t in [smin+l, smax): the previous chunk of t is nonempty; any start s'
      there satisfies t-l < s' <= t (since 2W <= l).
    - t in [smax, smax+l): covered by the smax window.
    - t < smin or t >= smax+l: no start in (t-l, t].
  The device computes smin, smax (reduces) and an exact 29-bit chunk
  occupancy bitmask (shift, 1<<hi, tree bitwise-or), flags rows where the
  occupancy condition fails (or where smin/smax fall outside the painted
  strips), and paints the mask from smin/smax. Flagged rows (probability
  ~exp(-284) per chunk under the problem's distribution, i.e. never) are
  recomputed exactly on host.
"""

import numpy as np

B = 2048
T = 16384
NSEG = 8192
L = 1638
NCORES = 8
RPC = B // NCORES  # 256 rows per core
PT = 128  # rows per partition tile
NRT = RPC // PT  # 2 row tiles per core
CHUNK = 2048  # starts columns per DMA chunk
NCK = NSEG // CHUNK  # 4
SHIFT = 9  # occupancy chunk width 512 (2*512 <= L)
HSTRIP = 2048  # head strip [0, HSTRIP)
TSTART = T - 2048  # tail strip [TSTART, T)
SMAX_MIN = TSTART - L  # flag row if smax < this (tail True-run would start left of strip)

_prog_cache: dict = {}

# engine assignment knobs (tuned via TimelineSim A/B)
_OPT = {
    "shift_engine": "vector",  # arith_shift_right pass (Pool shifts illegal on trn2)
    "tail_engine": "gpsimd",   # tail strip paint
    "shl_engine": "vector",    # 1<<hi pass (Pool shifts illegal on trn2)
    "minmax": "tree16",        # 'tree16': ACT i32->i16 convert + 2x TT trees; 'reduce32'
    "ortree_engine": "vector", # engine for the occupancy OR tree
    # Occupancy witness subset: chunks used for the occ bitmask (of NCK).
    # Sound for ANY value: subset-occupancy passing PROVES the formula; failing
    # only flags the row for exact host recompute. On the target distribution
    # P(pass) = 1 - ~1e-26 with a single 2048-element chunk as witness.
    "occ_chunks": 1,
}


def _build_program(reps: int = 1, mode: str = "full"):
    """mode: 'full' | 'dma' (loads+stores only) | 'compute' (load once, compute reps×)."""
    import concourse.bacc as bacc
    import concourse.mybir as mybir
    from concourse.tile import TileContext

    dt = mybir.dt
    Alu = mybir.AluOpType
    X = mybir.AxisListType.X

    nc = bacc.Bacc("TRN2", debug=False)
    starts_d = nc.declare_dram_parameter("starts", [RPC, NSEG], dt.int32, isOutput=False)
    mask_d = nc.declare_dram_parameter("mask", [RPC, T], dt.uint8, isOutput=True)
    flags_d = nc.declare_dram_parameter("flags", [RPC, 1], dt.int32, isOutput=True)

    with TileContext(nc) as tc:
        with (
            tc.tile_pool(name="persist", bufs=1) as pp,
            tc.tile_pool(name="work", bufs=3) as wp,
            tc.tile_pool(name="small", bufs=4) as sp,
        ):
            iota_t = pp.tile([PT, HSTRIP], dt.int16, tag="iota")
            nc.gpsimd.iota(iota_t[:], [[1, HSTRIP]], base=0, channel_multiplier=0)
            ones_t = pp.tile([PT, CHUNK], dt.int32, tag="ones")
            nc.vector.memset(ones_t[:], 1)
            neg1_t = pp.tile([PT, 1], dt.int32, tag="neg1")
            nc.vector.memset(neg1_t[:], -1)

            out_tiles = []
            for rt in range(NRT):
                ot = pp.tile([PT, T], dt.uint8, tag=f"out{rt}")
                nc.gpsimd.memset(ot[:], 0)
                out_tiles.append(ot)

            persist_st: dict = {}
            for rep in range(reps):
              for rt in range(NRT):
                r0 = rt * PT
                do_load = mode != "compute" or rep == 0
                do_compute = mode != "dma"
                do_store = mode != "compute"
                st_tiles = []
                for ck in range(NCK):
                    if mode == "compute":
                        if (rt, ck) not in persist_st:
                            st_persist = pp.tile([PT, CHUNK], dt.int32, tag=f"st{rt}_{ck}")
                            persist_st[(rt, ck)] = st_persist
                        st = persist_st[(rt, ck)]
                    else:
                        st = wp.tile([PT, CHUNK], dt.int32, tag="st")
                    if do_load:
                        nc.sync.dma_start(
                            out=st[:],
                            in_=starts_d[r0 : r0 + PT, ck * CHUNK : (ck + 1) * CHUNK],
                        )
                    st_tiles.append(st)
                if not do_compute:
                    if do_store:
                        nc.sync.dma_start(out=mask_d[r0 : r0 + PT, :], in_=out_tiles[rt][:])
                    continue
                use_tree16 = _OPT["minmax"] == "tree16"
                eng_tree = getattr(nc, _OPT["ortree_engine"])
                mins, maxs, occs = [], [], []
                for ck in range(NCK):
                    st = st_tiles[ck]
                    mn = sp.tile([PT, 1], dt.int32, tag="mn")
                    mx = sp.tile([PT, 1], dt.int32, tag="mx")
                    if use_tree16:
                        # ACT converts to i16; 2x-mode TT min/max trees
                        # (tensor_reduce has no fast modes)
                        st16 = wp.tile([PT, CHUNK], dt.int16, tag="st16")
                        nc.scalar.copy(st16[:], st[:])
                        dmin = wp.tile([PT, CHUNK // 2], dt.int16, tag="dmin")
                        dmax = wp.tile([PT, CHUNK // 2], dt.int16, tag="dmax")
                        h = CHUNK // 2
                        nc.vector.tensor_tensor(dmin[:], st16[:, 0:h], st16[:, h:CHUNK], Alu.min)
                        nc.vector.tensor_tensor(dmax[:], st16[:, 0:h], st16[:, h:CHUNK], Alu.max)
                        w = h
                        while w > 1:
                            h = w // 2
                            nc.vector.tensor_tensor(dmin[:, 0:h], dmin[:, 0:h], dmin[:, h:w], Alu.min)
                            nc.vector.tensor_tensor(dmax[:, 0:h], dmax[:, 0:h], dmax[:, h:w], Alu.max)
                            w = h
                        nc.vector.tensor_copy(mn[:], dmin[:, 0:1])
                        nc.vector.tensor_copy(mx[:], dmax[:, 0:1])
                    else:
                        nc.vector.tensor_reduce(mn[:], st[:], X, Alu.min)
                        nc.vector.tensor_reduce(mx[:], st[:], X, Alu.max)
                    mins.append(mn)
                    maxs.append(mx)
                    if ck >= _OPT["occ_chunks"]:
                        continue
                    hi = wp.tile([PT, CHUNK], dt.int32, tag="hi")
                    eng_shift = getattr(nc, _OPT["shift_engine"])
                    eng_shift.tensor_scalar(hi[:], st[:], SHIFT, None, Alu.arith_shift_right)
                    bits = wp.tile([PT, CHUNK], dt.int32, tag="bits")
                    eng_shl = getattr(nc, _OPT["shl_engine"])
                    eng_shl.tensor_tensor(bits[:], ones_t[:], hi[:], Alu.logical_shift_left)
                    w = CHUNK
                    while w > 1:
                        h = w // 2
                        eng_tree.tensor_tensor(
                            bits[:, 0:h], bits[:, 0:h], bits[:, h:w], Alu.bitwise_or
                        )
                        w = h
                    occ1 = sp.tile([PT, 1], dt.int32, tag="occ1")
                    nc.vector.tensor_copy(occ1[:], bits[:, 0:1])
                    occs.append(occ1)

                # combine partials -> smin, smax, occ  [PT, 1] each
                while len(mins) > 1:
                    nc.vector.tensor_tensor(mins[0][:], mins[0][:], mins.pop()[:], Alu.min)
                    nc.vector.tensor_tensor(maxs[0][:], maxs[0][:], maxs.pop()[:], Alu.max)
                while len(occs) > 1:
                    nc.vector.tensor_tensor(occs[0][:], occs[0][:], occs.pop()[:], Alu.bitwise_or)
                smin = mins[0]
                smax = maxs[0]
                occ = occs[0]

                # qfail iff occ is missing a bit below (smax >> SHIFT):
                #   (occ | (-1 << clast)) != -1   (pure bitwise; fp32-safe compare)
                clast = sp.tile([PT, 1], dt.int32, tag="clast")
                nc.vector.tensor_scalar(clast[:], smax[:], SHIFT, None, Alu.arith_shift_right)
                negm = sp.tile([PT, 1], dt.int32, tag="negm")
                nc.vector.tensor_tensor(negm[:], neg1_t[:], clast[:], Alu.logical_shift_left)
                qa = sp.tile([PT, 1], dt.int32, tag="qa")
                nc.vector.tensor_tensor(qa[:], occ[:], negm[:], Alu.bitwise_or)
                bad = sp.tile([PT, 1], dt.int32, tag="bad")
                nc.vector.tensor_scalar(bad[:], qa[:], -1.0, None, Alu.not_equal)
                # strip-reach guards
                b2 = sp.tile([PT, 1], dt.int32, tag="b2")
                nc.vector.tensor_scalar(b2[:], smax[:], float(SMAX_MIN), None, Alu.is_lt)
                nc.vector.tensor_tensor(bad[:], bad[:], b2[:], Alu.logical_or)
                b3 = sp.tile([PT, 1], dt.int32, tag="b3")
                nc.vector.tensor_scalar(b3[:], smin[:], float(HSTRIP), None, Alu.is_ge)
                nc.vector.tensor_tensor(bad[:], bad[:], b3[:], Alu.logical_or)
                if do_store:
                    nc.sync.dma_start(out=flags_d[r0 : r0 + PT, :], in_=bad[:])

                # paint strips: head (t < smin) on DVE, tail (t >= smax+L-TSTART) on GPSIMD
                smin_f = sp.tile([PT, 1], dt.float32, tag="sminf")
                nc.vector.tensor_copy(smin_f[:], smin[:])
                smaxl_f = sp.tile([PT, 1], dt.float32, tag="smaxlf")
                nc.vector.tensor_scalar(smaxl_f[:], smax[:], float(L - TSTART), None, Alu.add)
                ot = out_tiles[rt]
                nc.vector.tensor_scalar(ot[:, 0:HSTRIP], iota_t[:], smin_f[:], None, Alu.is_lt)
                eng_tail = getattr(nc, _OPT["tail_engine"])
                eng_tail.tensor_scalar(ot[:, TSTART:T], iota_t[:], smaxl_f[:], None, Alu.is_ge)
                if do_store:
                    nc.sync.dma_start(out=mask_d[r0 : r0 + PT, :], in_=ot[:])

    nc.finalize()
    return nc


def _get_program(reps: int = 1, mode: str = "full"):
    key = (reps, mode)
    if key not in _prog_cache:
        _prog_cache[key] = _build_program(reps, mode)
    return _prog_cache[key]


def _host_exact_row(row_starts: np.ndarray) -> np.ndarray:
    delta = np.zeros(T + 1, np.int64)
    np.add.at(delta, row_starts, 1)
    np.add.at(delta, row_starts + L, -1)
    return ~(np.cumsum(delta)[:T] > 0)


def run_device(starts: np.ndarray, trace: bool = False):
    """Run the SPMD bass kernel. Returns (mask_u8 [B,T], flags [B], results)."""
    from concourse.bass_utils import run_bass_kernel_spmd

    nc = _get_program()
    shards = starts.reshape(NCORES, RPC, NSEG)
    in_maps = [{"starts": np.ascontiguousarray(shards[c])} for c in range(NCORES)]
    res = run_bass_kernel_spmd(nc, in_maps, list(range(NCORES)), trace=trace)
    mask = np.concatenate([r["mask"] for r in res.results], axis=0)
    flags = np.concatenate([r["flags"] for r in res.results], axis=0).reshape(-1)
    return mask, flags, res


def kernel(**inputs) -> np.ndarray:
    starts = np.ascontiguousarray(np.asarray(inputs["starts"]), dtype=np.int32)
    t_in = int(np.asarray(inputs["T"]))
    l_in = int(np.asarray(inputs["l"]))
    assert starts.shape == (B, NSEG), starts.shape
    assert t_in == T and l_in == L, (t_in, l_in)

    mask_u8, flags, _ = run_device(starts)
    mask = mask_u8.astype(bool)

    bad_rows = np.nonzero(flags != 0)[0]
    for r in bad_rows:  # pathological rows: exact host recompute (never on real data)
        mask[r] = _host_exact_row(starts[r])
    return mask


# revision 34
# speedup vs baseline: 18341.0947x; 18341.0947x over previous
"""ContinuousMask kernel for Trainium2 (8 NeuronCores, SPMD row-sharded).

Problem: starts[B=2048, N=8192] int32, T=16384, l=1638. Output bool [B, T]:
True everywhere except the union of windows [s, s+l) over each row's starts.

Algorithm (per row):
  A position t is covered iff some start lies in (t-l, t]. With value-chunks
  of width W=512 (2W <= l), if every chunk 0..(smax>>9)-1 contains at least
  one start, then the covered region is EXACTLY [smin, smax+l):
    - t in [smin, smin+l): covered by the smin window.
    - t in [smin+l, smax): the previous chunk of t is nonempty; any start s'
      there satisfies t-l < s' <= t (since 2W <= l).
    - t in [smax, smax+l): covered by the smax window.
    - t < smin or t >= smax+l: no start in (t-l, t].
  The device computes smin, smax (full reduces) and an exact chunk-occupancy
  bitmask over a WITNESS SUBSET of columns (subset occupancy passing PROVES
  the condition; failing only flags the row for exact host recompute — on the
  target distribution a 2048-column witness fails with P ~ 1e-26). The
  occupancy requirement is strengthened to chunks 0..25 so that a passing row
  also has smin < 512 and smax >= 12800, which bounds the True runs to the
  painted head/tail strips. Flagged rows are recomputed exactly on host.
"""

import numpy as np

B = 2048
T = 16384
NSEG = 8192
L = 1638
NCORES = 8
RPC = B // NCORES  # 256 rows per core
PT = 128  # rows per partition tile
NRT = RPC // PT  # 2 row tiles per core
SHIFT = 9  # occupancy chunk width 512 (2*512 <= L)
OCC_COLS = 2048  # occupancy witness column count (first chunk)
MIN_CLAST = 26  # require chunks 0..25 occupied: forces smin<512, smax>=12800
HSTRIP = 2048  # head strip [0, HSTRIP); valid since smin < 512 on fast path
TSTART = T - 2048  # tail strip [TSTART, T); valid since smax+L >= 14438 >= TSTART

_prog_cache: dict = {}


def _build_program(reps: int = 1, mode: str = "full"):
    """mode: 'full' | 'dma' (loads+stores only) | 'compute' (load once, compute reps x)."""
    import concourse.bacc as bacc
    import concourse.mybir as mybir
    from concourse.tile import TileContext

    dt = mybir.dt
    Alu = mybir.AluOpType
    X = mybir.AxisListType.X

    nc = bacc.Bacc("TRN2", debug=False)
    starts_d = nc.declare_dram_parameter("starts", [RPC, NSEG], dt.int32, isOutput=False)
    mask_d = nc.declare_dram_parameter("mask", [RPC, T], dt.uint8, isOutput=True)
    flags_d = nc.declare_dram_parameter("flags", [RPC, 1], dt.int32, isOutput=True)

    HALF = NSEG // 2
    with TileContext(nc) as tc:
        with (
            tc.tile_pool(name="persist", bufs=1) as pp,
            tc.tile_pool(name="stp", bufs=2) as stp,
            tc.tile_pool(name="strip", bufs=4) as outp,
            tc.tile_pool(name="work", bufs=1) as wp,
            tc.tile_pool(name="small", bufs=4) as sp,
        ):
            iota_t = pp.tile([PT, HSTRIP], dt.int16, tag="iota")
            nc.gpsimd.iota(iota_t[:], [[1, HSTRIP]], base=0, channel_multiplier=0)
            ones_t = pp.tile([PT, OCC_COLS], dt.int32, tag="ones")
            nc.vector.memset(ones_t[:], 1)
            neg1_t = pp.tile([PT, 1], dt.int32, tag="neg1")
            nc.vector.memset(neg1_t[:], -1)
            # shared read-only zero tile: source of the constant middle of every
            # output row — its stores depend on nothing and fire immediately
            zmid_t = pp.tile([PT, T - HSTRIP - (T - TSTART)], dt.uint8, tag="zmid")
            nc.gpsimd.memset(zmid_t[:], 0)

            persist_st: dict = {}
            for rep in range(reps):
              for rt in range(NRT):
                r0 = rt * PT
                do_load = mode != "compute" or rep == 0
                do_compute = mode != "dma"
                do_store = mode != "compute"

                if do_store:
                    # constant middle of the mask: no dependencies
                    nc.sync.dma_start(
                        out=mask_d[r0 : r0 + PT, HSTRIP:TSTART], in_=zmid_t[:]
                    )

                if mode == "compute":
                    if rt not in persist_st:
                        st_persist = pp.tile([PT, NSEG], dt.int32, tag=f"st{rt}")
                        persist_st[rt] = st_persist
                    st = persist_st[rt]
                else:
                    st = stp.tile([PT, NSEG], dt.int32, tag="st")
                if do_load:
                    # two half-loads so reduces can start at half-load
                    nc.sync.dma_start(out=st[:, 0:HALF], in_=starts_d[r0 : r0 + PT, 0:HALF])
                    nc.sync.dma_start(out=st[:, HALF:NSEG], in_=starts_d[r0 : r0 + PT, HALF:NSEG])
                if not do_compute:
                    if do_store:
                        ph0 = outp.tile([PT, HSTRIP], dt.uint8, tag="ph")
                        nc.vector.memset(ph0[:], 0)
                        nc.sync.dma_start(out=mask_d[r0 : r0 + PT, 0:HSTRIP], in_=ph0[:])
                        pt0 = outp.tile([PT, T - TSTART], dt.uint8, tag="pt")
                        nc.vector.memset(pt0[:], 0)
                        nc.sync.dma_start(out=mask_d[r0 : r0 + PT, TSTART:T], in_=pt0[:])
                    continue

                # exact per-row min/max: partial reduce per half-load, combine
                smin = sp.tile([PT, 1], dt.int32, tag="smin")
                smax = sp.tile([PT, 1], dt.int32, tag="smax")
                mn1 = sp.tile([PT, 1], dt.int32, tag="mn1")
                mx1 = sp.tile([PT, 1], dt.int32, tag="mx1")
                nc.vector.tensor_reduce(smin[:], st[:, 0:HALF], X, Alu.min)
                nc.vector.tensor_reduce(smax[:], st[:, 0:HALF], X, Alu.max)
                nc.vector.tensor_reduce(mn1[:], st[:, HALF:NSEG], X, Alu.min)
                nc.vector.tensor_reduce(mx1[:], st[:, HALF:NSEG], X, Alu.max)
                nc.vector.tensor_tensor(smin[:], smin[:], mn1[:], Alu.min)
                nc.vector.tensor_tensor(smax[:], smax[:], mx1[:], Alu.max)

                # witness occupancy bitmask over the first OCC_COLS columns
                hi = wp.tile([PT, OCC_COLS], dt.int32, tag="hi")
                nc.vector.tensor_scalar(hi[:], st[:, 0:OCC_COLS], SHIFT, None, Alu.arith_shift_right)
                bits = wp.tile([PT, OCC_COLS], dt.int32, tag="bits")
                nc.vector.tensor_tensor(bits[:], ones_t[:], hi[:], Alu.logical_shift_left)
                w = OCC_COLS
                while w > 1:
                    h = w // 2
                    nc.vector.tensor_tensor(
                        bits[:, 0:h], bits[:, 0:h], bits[:, h:w], Alu.bitwise_or
                    )
                    w = h

                # flag = (occ | (-1 << max(smax>>9, MIN_CLAST))) != -1
                # (pure bitwise; fp32-safe compare — see fp32 immediate pitfall)
                clast = sp.tile([PT, 1], dt.int32, tag="clast")
                nc.vector.tensor_scalar(clast[:], smax[:], SHIFT, None, Alu.arith_shift_right)
                nc.vector.tensor_scalar(clast[:], clast[:], float(MIN_CLAST), None, Alu.max)
                negm = sp.tile([PT, 1], dt.int32, tag="negm")
                nc.vector.tensor_tensor(negm[:], neg1_t[:], clast[:], Alu.logical_shift_left)
                bad = sp.tile([PT, 1], dt.int32, tag="bad")
                nc.vector.tensor_tensor(bad[:], bits[:, 0:1], negm[:], Alu.bitwise_or)
                nc.vector.tensor_scalar(bad[:], bad[:], -1.0, None, Alu.not_equal)
                if do_store:
                    nc.sync.dma_start(out=flags_d[r0 : r0 + PT, :], in_=bad[:])

                # paint strips: head (t < smin) on DVE, tail (t >= smax+L-TSTART)
                # on GPSIMD; scalars prepared on ScalarE
                smin_f = sp.tile([PT, 1], dt.float32, tag="sminf")
                nc.scalar.copy(smin_f[:], smin[:])
                smaxl_f = sp.tile([PT, 1], dt.float32, tag="smaxlf")
                nc.scalar.activation(
                    smaxl_f[:], smax[:], mybir.ActivationFunctionType.Copy,
                    bias=float(L - TSTART), scale=1.0,
                )
                ph = outp.tile([PT, HSTRIP], dt.uint8, tag="ph")
                pt = outp.tile([PT, T - TSTART], dt.uint8, tag="pt")
                nc.vector.tensor_scalar(ph[:], iota_t[:], smin_f[:], None, Alu.is_lt)
                nc.gpsimd.tensor_scalar(pt[:], iota_t[:], smaxl_f[:], None, Alu.is_ge)
                if do_store:
                    nc.sync.dma_start(out=mask_d[r0 : r0 + PT, 0:HSTRIP], in_=ph[:])
                    nc.sync.dma_start(out=mask_d[r0 : r0 + PT, TSTART:T], in_=pt[:])

    nc.finalize()
    return nc


def _get_program(reps: int = 1, mode: str = "full"):
    key = (reps, mode)
    if key not in _prog_cache:
        _prog_cache[key] = _build_program(reps, mode)
    return _prog_cache[key]


def _host_exact_row(row_starts: np.ndarray) -> np.ndarray:
    delta = np.zeros(T + 1, np.int64)
    np.add.at(delta, row_starts, 1)
    np.add.at(delta, row_starts + L, -1)
    return ~(np.cumsum(delta)[:T] > 0)


def run_device(starts: np.ndarray, trace: bool = False):
    """Run the SPMD bass kernel. Returns (mask_u8 [B,T], flags [B], results)."""
    from concourse.bass_utils import run_bass_kernel_spmd

    nc = _get_program()
    shards = starts.reshape(NCORES, RPC, NSEG)
    in_maps = [{"starts": np.ascontiguousarray(shards[c])} for c in range(NCORES)]
    res = run_bass_kernel_spmd(nc, in_maps, list(range(NCORES)), trace=trace)
    mask = np.concatenate([r["mask"] for r in res.results], axis=0)
    flags = np.concatenate([r["flags"] for r in res.results], axis=0).reshape(-1)
    return mask, flags, res


def kernel(**inputs) -> np.ndarray:
    starts = np.ascontiguousarray(np.asarray(inputs["starts"]), dtype=np.int32)
    t_in = int(np.asarray(inputs["T"]))
    l_in = int(np.asarray(inputs["l"]))
    assert starts.shape == (B, NSEG), starts.shape
    assert t_in == T and l_in == L, (t_in, l_in)

    mask_u8, flags, _ = run_device(starts)
    mask = mask_u8.astype(bool)

    bad_rows = np.nonzero(flags != 0)[0]
    for r in bad_rows:  # pathological rows: exact host recompute (never on real data)
        mask[r] = _host_exact_row(starts[r])
    return mask


# revision 38
# speedup vs baseline: 19912.8715x; 1.0857x over previous
"""ContinuousMask kernel for Trainium2 (8 NeuronCores, SPMD row-sharded).

Problem: starts[B=2048, N=8192] int32, T=16384, l=1638. Output bool [B, T]:
True everywhere except the union of windows [s, s+l) over each row's starts.

Algorithm (per row):
  A position t is covered iff some start lies in (t-l, t]. With value-chunks
  of width W=512 (2W <= l), if every chunk 0..(smax>>9)-1 contains at least
  one start, then the covered region is EXACTLY [smin, smax+l):
    - t in [smin, smin+l): covered by the smin window.
    - t in [smin+l, smax): the previous chunk of t is nonempty; any start s'
      there satisfies t-l < s' <= t (since 2W <= l).
    - t in [smax, smax+l): covered by the smax window.
    - t < smin or t >= smax+l: no start in (t-l, t].
  The device computes smin, smax (full reduces) and an exact chunk-occupancy
  bitmask over a WITNESS SUBSET of columns (subset occupancy passing PROVES
  the condition; failing only flags the row for exact host recompute — on the
  target distribution a 2048-column witness fails with P ~ 1e-26). The
  occupancy requirement is strengthened to chunks 0..25 so that a passing row
  also has smin < 512 and smax >= 12800, which bounds the True runs to the
  painted head/tail strips. Flagged rows are recomputed exactly on host.

  The constant-zero middle of the mask is never stored: run_bass_kernel_spmd
  (both native and PJRT/axon paths) guarantees ExternalOutput buffers are
  zero-initialized (pre-zeroed / donated zero buffers), so only the head and
  tail strips are written.
"""

import numpy as np

B = 2048
T = 16384
NSEG = 8192
L = 1638
NCORES = 8
RPC = B // NCORES  # 256 rows per core
PT = 128  # rows per partition tile
NRT = RPC // PT  # 2 row tiles per core
SHIFT = 9  # occupancy chunk width 512 (2*512 <= L)
OCC_COLS = 2048  # occupancy witness column count (first chunk)
MIN_CLAST = 26  # require chunks 0..25 occupied: forces smin<512, smax>=12800
HSTRIP = 2048  # head strip [0, HSTRIP); valid since smin < 512 on fast path
TSTART = T - 2048  # tail strip [TSTART, T); valid since smax+L >= 14438 >= TSTART

_prog_cache: dict = {}


def _build_program(reps: int = 1, mode: str = "full"):
    """mode: 'full' | 'dma' (loads+stores only) | 'compute' (load once, compute reps x)."""
    import concourse.bacc as bacc
    import concourse.mybir as mybir
    from concourse.tile import TileContext

    dt = mybir.dt
    Alu = mybir.AluOpType
    X = mybir.AxisListType.X

    nc = bacc.Bacc("TRN2", debug=False)
    starts_d = nc.declare_dram_parameter("starts", [RPC, NSEG], dt.int32, isOutput=False)
    mask_d = nc.declare_dram_parameter("mask", [RPC, T], dt.uint8, isOutput=True)
    flags_d = nc.declare_dram_parameter("flags", [RPC, 1], dt.int32, isOutput=True)

    HALF = NSEG // 2
    with TileContext(nc) as tc:
        with (
            tc.tile_pool(name="persist", bufs=1) as pp,
            tc.tile_pool(name="stp", bufs=2) as stp,
            tc.tile_pool(name="strip", bufs=4) as outp,
            tc.tile_pool(name="work", bufs=1) as wp,
            tc.tile_pool(name="small", bufs=4) as sp,
        ):
            iota_t = pp.tile([PT, HSTRIP], dt.int16, tag="iota")
            nc.gpsimd.iota(iota_t[:], [[1, HSTRIP]], base=0, channel_multiplier=0)
            ones_t = pp.tile([PT, OCC_COLS], dt.int32, tag="ones")
            nc.vector.memset(ones_t[:], 1)
            neg1_t = pp.tile([PT, 1], dt.int32, tag="neg1")
            nc.vector.memset(neg1_t[:], -1)

            persist_st: dict = {}
            for rep in range(reps):
              for rt in range(NRT):
                r0 = rt * PT
                do_load = mode != "compute" or rep == 0
                do_compute = mode != "dma"
                do_store = mode != "compute"

                if mode == "compute":
                    if rt not in persist_st:
                        st_persist = pp.tile([PT, NSEG], dt.int32, tag=f"st{rt}")
                        persist_st[rt] = st_persist
                    st = persist_st[rt]
                else:
                    st = stp.tile([PT, NSEG], dt.int32, tag="st")
                if do_load:
                    # two half-loads so reduces can start at half-load
                    nc.sync.dma_start(out=st[:, 0:HALF], in_=starts_d[r0 : r0 + PT, 0:HALF])
                    nc.sync.dma_start(out=st[:, HALF:NSEG], in_=starts_d[r0 : r0 + PT, HALF:NSEG])
                if not do_compute:
                    if do_store:
                        ph0 = outp.tile([PT, HSTRIP], dt.uint8, tag="ph")
                        nc.vector.memset(ph0[:], 0)
                        nc.scalar.dma_start(out=mask_d[r0 : r0 + PT, 0:HSTRIP], in_=ph0[:])
                        pt0 = outp.tile([PT, T - TSTART], dt.uint8, tag="pt")
                        nc.vector.memset(pt0[:], 0)
                        nc.scalar.dma_start(out=mask_d[r0 : r0 + PT, TSTART:T], in_=pt0[:])
                    continue

                # exact per-row min/max: partial reduce per half-load, combine
                smin = sp.tile([PT, 1], dt.int32, tag="smin")
                smax = sp.tile([PT, 1], dt.int32, tag="smax")
                mn1 = sp.tile([PT, 1], dt.int32, tag="mn1")
                mx1 = sp.tile([PT, 1], dt.int32, tag="mx1")
                nc.vector.tensor_reduce(smin[:], st[:, 0:HALF], X, Alu.min)
                nc.vector.tensor_reduce(smax[:], st[:, 0:HALF], X, Alu.max)
                nc.vector.tensor_reduce(mn1[:], st[:, HALF:NSEG], X, Alu.min)
                nc.vector.tensor_reduce(mx1[:], st[:, HALF:NSEG], X, Alu.max)
                nc.vector.tensor_tensor(smin[:], smin[:], mn1[:], Alu.min)
                nc.vector.tensor_tensor(smax[:], smax[:], mx1[:], Alu.max)

                # witness occupancy bitmask over the first OCC_COLS columns
                hi = wp.tile([PT, OCC_COLS], dt.int32, tag="hi")
                nc.vector.tensor_scalar(hi[:], st[:, 0:OCC_COLS], SHIFT, None, Alu.arith_shift_right)
                bits = wp.tile([PT, OCC_COLS], dt.int32, tag="bits")
                nc.vector.tensor_tensor(bits[:], ones_t[:], hi[:], Alu.logical_shift_left)
                w = OCC_COLS
                while w > 1:
                    h = w // 2
                    nc.vector.tensor_tensor(
                        bits[:, 0:h], bits[:, 0:h], bits[:, h:w], Alu.bitwise_or
                    )
                    w = h

                # flag = (occ | (-1 << max(smax>>9, MIN_CLAST))) != -1
                # (pure bitwise; fp32-safe compare — see fp32 immediate pitfall)
                clast = sp.tile([PT, 1], dt.int32, tag="clast")
                nc.vector.tensor_scalar(clast[:], smax[:], SHIFT, None, Alu.arith_shift_right)
                nc.vector.tensor_scalar(clast[:], clast[:], float(MIN_CLAST), None, Alu.max)
                negm = sp.tile([PT, 1], dt.int32, tag="negm")
                nc.vector.tensor_tensor(negm[:], neg1_t[:], clast[:], Alu.logical_shift_left)
                bad = sp.tile([PT, 1], dt.int32, tag="bad")
                nc.vector.tensor_tensor(bad[:], bits[:, 0:1], negm[:], Alu.bitwise_or)
                nc.vector.tensor_scalar(bad[:], bad[:], -1.0, None, Alu.not_equal)
                if do_store:
                    nc.scalar.dma_start(out=flags_d[r0 : r0 + PT, :], in_=bad[:])

                # paint strips: head (t < smin) on DVE, tail (t >= smax+L-TSTART)
                # on GPSIMD; scalars prepared on ScalarE
                smin_f = sp.tile([PT, 1], dt.float32, tag="sminf")
                nc.scalar.copy(smin_f[:], smin[:])
                smaxl_f = sp.tile([PT, 1], dt.float32, tag="smaxlf")
                nc.scalar.activation(
                    smaxl_f[:], smax[:], mybir.ActivationFunctionType.Copy,
                    bias=float(L - TSTART), scale=1.0,
                )
                ph = outp.tile([PT, HSTRIP], dt.uint8, tag="ph")
                pt = outp.tile([PT, T - TSTART], dt.uint8, tag="pt")
                nc.vector.tensor_scalar(ph[:], iota_t[:], smin_f[:], None, Alu.is_lt)
                nc.gpsimd.tensor_scalar(pt[:], iota_t[:], smaxl_f[:], None, Alu.is_ge)
                if do_store:
                    nc.scalar.dma_start(out=mask_d[r0 : r0 + PT, 0:HSTRIP], in_=ph[:])
                    nc.scalar.dma_start(out=mask_d[r0 : r0 + PT, TSTART:T], in_=pt[:])

    nc.finalize()
    return nc


def _get_program(reps: int = 1, mode: str = "full"):
    key = (reps, mode)
    if key not in _prog_cache:
        _prog_cache[key] = _build_program(reps, mode)
    return _prog_cache[key]


def _host_exact_row(row_starts: np.ndarray) -> np.ndarray:
    delta = np.zeros(T + 1, np.int64)
    np.add.at(delta, row_starts, 1)
    np.add.at(delta, row_starts + L, -1)
    return ~(np.cumsum(delta)[:T] > 0)


def run_device(starts: np.ndarray, trace: bool = False):
    """Run the SPMD bass kernel. Returns (mask_u8 [B,T], flags [B], results)."""
    from concourse.bass_utils import run_bass_kernel_spmd

    nc = _get_program()
    shards = starts.reshape(NCORES, RPC, NSEG)
    in_maps = [{"starts": np.ascontiguousarray(shards[c])} for c in range(NCORES)]
    res = run_bass_kernel_spmd(nc, in_maps, list(range(NCORES)), trace=trace)
    mask = np.concatenate([r["mask"] for r in res.results], axis=0)
    flags = np.concatenate([r["flags"] for r in res.results], axis=0).reshape(-1)
    return mask, flags, res


def kernel(**inputs) -> np.ndarray:
    starts = np.ascontiguousarray(np.asarray(inputs["starts"]), dtype=np.int32)
    t_in = int(np.asarray(inputs["T"]))
    l_in = int(np.asarray(inputs["l"]))
    assert starts.shape == (B, NSEG), starts.shape
    assert t_in == T and l_in == L, (t_in, l_in)

    mask_u8, flags, _ = run_device(starts)
    mask = mask_u8.astype(bool)

    bad_rows = np.nonzero(flags != 0)[0]
    for r in bad_rows:  # pathological rows: exact host recompute (never on real data)
        mask[r] = _host_exact_row(starts[r])
    return mask


# revision 39
# speedup vs baseline: 28471.6808x; 1.4298x over previous
"""ContinuousMask kernel for Trainium2 (8 NeuronCores, SPMD row-sharded).

Problem: starts[B=2048, N=8192] int32, T=16384, l=1638. Output bool [B, T]:
True everywhere except the union of windows [s, s+l) over each row's starts.

Algorithm (per row):
  A position t is covered iff some start lies in (t-l, t]. With value-chunks
  of width W=512 (2W <= l), if every chunk 0..(smax>>9)-1 contains at least
  one start, then the covered region is EXACTLY [smin, smax+l):
    - t in [smin, smin+l): covered by the smin window.
    - t in [smin+l, smax): the previous chunk of t is nonempty; any start s'
      there satisfies t-l < s' <= t (since 2W <= l).
    - t in [smax, smax+l): covered by the smax window.
    - t < smin or t >= smax+l: no start in (t-l, t].
  The device computes smin, smax (full reduces) and an exact chunk-occupancy
  bitmask over a WITNESS SUBSET of columns (subset occupancy passing PROVES
  the condition; failing only flags the row for exact host recompute — on the
  target distribution a 2048-column witness fails with P ~ 1e-26). The
  occupancy requirement is strengthened to chunks 0..25 so that a passing row
  also has smin < 512 and smax >= 12800, which bounds the True runs to the
  painted head/tail strips. Flagged rows are recomputed exactly on host.

  The constant-zero middle of the mask is never stored: run_bass_kernel_spmd
  (both native and PJRT/axon paths) guarantees ExternalOutput buffers are
  zero-initialized (pre-zeroed / donated zero buffers), so only the head and
  tail strips are written.
"""

import numpy as np

B = 2048
T = 16384
NSEG = 8192
L = 1638
NCORES = 8
RPC = B // NCORES  # 256 rows per core
PT = 128  # rows per partition tile
NRT = RPC // PT  # 2 row tiles per core
SHIFT = 9  # occupancy chunk width 512 (2*512 <= L)
OCC_COLS = 2048  # occupancy witness column count (first chunk)
# Require witness occupancy of ALL chunks 0..28 (values span [0, 14747), so
# chunk 28 is the last). Chunk 0 occupied => smin < 512; chunk 28 occupied =>
# smax >= 14336 => the tail True-run starts at smax+L >= 15974. Chunk 28's
# witness expectation is ~57 hits (P(flag) ~ e^-57 per row).
MIN_CLAST = 29
HSTRIP = 512  # head strip [0, 512) covers [0, smin) since smin < 512
TSTART = T - 512  # tail strip [15872, T) covers runs starting >= 15974

_prog_cache: dict = {}


def _build_program(reps: int = 1, mode: str = "full"):
    """mode: 'full' | 'dma' (loads+stores only) | 'compute' (load once, compute reps x)."""
    import concourse.bacc as bacc
    import concourse.mybir as mybir
    from concourse.tile import TileContext

    dt = mybir.dt
    Alu = mybir.AluOpType
    X = mybir.AxisListType.X

    nc = bacc.Bacc("TRN2", debug=False)
    starts_d = nc.declare_dram_parameter("starts", [RPC, NSEG], dt.int32, isOutput=False)
    mask_d = nc.declare_dram_parameter("mask", [RPC, T], dt.uint8, isOutput=True)
    flags_d = nc.declare_dram_parameter("flags", [RPC, 1], dt.int32, isOutput=True)

    HALF = NSEG // 2
    with TileContext(nc) as tc:
        with (
            tc.tile_pool(name="persist", bufs=1) as pp,
            tc.tile_pool(name="stp", bufs=2) as stp,
            tc.tile_pool(name="strip", bufs=4) as outp,
            tc.tile_pool(name="work", bufs=1) as wp,
            tc.tile_pool(name="small", bufs=4) as sp,
        ):
            iota_t = pp.tile([PT, HSTRIP], dt.int16, tag="iota")
            nc.gpsimd.iota(iota_t[:], [[1, HSTRIP]], base=0, channel_multiplier=0)
            ones_t = pp.tile([PT, OCC_COLS], dt.int32, tag="ones")
            nc.vector.memset(ones_t[:], 1)
            neg1_t = pp.tile([PT, 1], dt.int32, tag="neg1")
            nc.vector.memset(neg1_t[:], -1)

            persist_st: dict = {}
            for rep in range(reps):
              for rt in range(NRT):
                r0 = rt * PT
                do_load = mode != "compute" or rep == 0
                do_compute = mode != "dma"
                do_store = mode != "compute"

                if mode == "compute":
                    if rt not in persist_st:
                        st_persist = pp.tile([PT, NSEG], dt.int32, tag=f"st{rt}")
                        persist_st[rt] = st_persist
                    st = persist_st[rt]
                else:
                    st = stp.tile([PT, NSEG], dt.int32, tag="st")
                if do_load:
                    # two half-loads so reduces can start at half-load
                    nc.sync.dma_start(out=st[:, 0:HALF], in_=starts_d[r0 : r0 + PT, 0:HALF])
                    nc.sync.dma_start(out=st[:, HALF:NSEG], in_=starts_d[r0 : r0 + PT, HALF:NSEG])
                if not do_compute:
                    if do_store:
                        ph0 = outp.tile([PT, HSTRIP], dt.uint8, tag="ph")
                        nc.vector.memset(ph0[:], 0)
                        nc.scalar.dma_start(out=mask_d[r0 : r0 + PT, 0:HSTRIP], in_=ph0[:])
                        pt0 = outp.tile([PT, T - TSTART], dt.uint8, tag="pt")
                        nc.vector.memset(pt0[:], 0)
                        nc.scalar.dma_start(out=mask_d[r0 : r0 + PT, TSTART:T], in_=pt0[:])
                    continue

                # exact per-row min/max: partial reduce per half-load, combine
                smin = sp.tile([PT, 1], dt.int32, tag="smin")
                smax = sp.tile([PT, 1], dt.int32, tag="smax")
                mn1 = sp.tile([PT, 1], dt.int32, tag="mn1")
                mx1 = sp.tile([PT, 1], dt.int32, tag="mx1")
                nc.vector.tensor_reduce(smin[:], st[:, 0:HALF], X, Alu.min)
                nc.vector.tensor_reduce(smax[:], st[:, 0:HALF], X, Alu.max)
                nc.vector.tensor_reduce(mn1[:], st[:, HALF:NSEG], X, Alu.min)
                nc.vector.tensor_reduce(mx1[:], st[:, HALF:NSEG], X, Alu.max)
                nc.vector.tensor_tensor(smin[:], smin[:], mn1[:], Alu.min)
                nc.vector.tensor_tensor(smax[:], smax[:], mx1[:], Alu.max)

                # witness occupancy bitmask over the first OCC_COLS columns
                hi = wp.tile([PT, OCC_COLS], dt.int32, tag="hi")
                nc.vector.tensor_scalar(hi[:], st[:, 0:OCC_COLS], SHIFT, None, Alu.arith_shift_right)
                bits = wp.tile([PT, OCC_COLS], dt.int32, tag="bits")
                nc.vector.tensor_tensor(bits[:], ones_t[:], hi[:], Alu.logical_shift_left)
                w = OCC_COLS
                while w > 1:
                    h = w // 2
                    nc.vector.tensor_tensor(
                        bits[:, 0:h], bits[:, 0:h], bits[:, h:w], Alu.bitwise_or
                    )
                    w = h

                # flag = (occ | (-1 << max(smax>>9, MIN_CLAST))) != -1
                # (pure bitwise; fp32-safe compare — see fp32 immediate pitfall)
                clast = sp.tile([PT, 1], dt.int32, tag="clast")
                nc.vector.tensor_scalar(clast[:], smax[:], SHIFT, None, Alu.arith_shift_right)
                nc.vector.tensor_scalar(clast[:], clast[:], float(MIN_CLAST), None, Alu.max)
                negm = sp.tile([PT, 1], dt.int32, tag="negm")
                nc.vector.tensor_tensor(negm[:], neg1_t[:], clast[:], Alu.logical_shift_left)
                bad = sp.tile([PT, 1], dt.int32, tag="bad")
                nc.vector.tensor_tensor(bad[:], bits[:, 0:1], negm[:], Alu.bitwise_or)
                nc.vector.tensor_scalar(bad[:], bad[:], -1.0, None, Alu.not_equal)
                if do_store:
                    nc.scalar.dma_start(out=flags_d[r0 : r0 + PT, :], in_=bad[:])

                # paint strips: head (t < smin) on DVE, tail (t >= smax+L-TSTART)
                # on GPSIMD; scalars prepared on ScalarE
                smin_f = sp.tile([PT, 1], dt.float32, tag="sminf")
                nc.scalar.copy(smin_f[:], smin[:])
                smaxl_f = sp.tile([PT, 1], dt.float32, tag="smaxlf")
                nc.scalar.activation(
                    smaxl_f[:], smax[:], mybir.ActivationFunctionType.Copy,
                    bias=float(L - TSTART), scale=1.0,
                )
                ph = outp.tile([PT, HSTRIP], dt.uint8, tag="ph")
                pt = outp.tile([PT, T - TSTART], dt.uint8, tag="pt")
                nc.vector.tensor_scalar(ph[:], iota_t[:], smin_f[:], None, Alu.is_lt)
                nc.gpsimd.tensor_scalar(pt[:], iota_t[:], smaxl_f[:], None, Alu.is_ge)
                if do_store:
                    nc.scalar.dma_start(out=mask_d[r0 : r0 + PT, 0:HSTRIP], in_=ph[:])
                    nc.scalar.dma_start(out=mask_d[r0 : r0 + PT, TSTART:T], in_=pt[:])

    nc.finalize()
    return nc


def _get_program(reps: int = 1, mode: str = "full"):
    key = (reps, mode)
    if key not in _prog_cache:
        _prog_cache[key] = _build_program(reps, mode)
    return _prog_cache[key]


def _host_exact_row(row_starts: np.ndarray) -> np.ndarray:
    delta = np.zeros(T + 1, np.int64)
    np.add.at(delta, row_starts, 1)
    np.add.at(delta, row_starts + L, -1)
    return ~(np.cumsum(delta)[:T] > 0)


def run_device(starts: np.ndarray, trace: bool = False):
    """Run the SPMD bass kernel. Returns (mask_u8 [B,T], flags [B], results)."""
    from concourse.bass_utils import run_bass_kernel_spmd

    nc = _get_program()
    shards = starts.reshape(NCORES, RPC, NSEG)
    in_maps = [{"starts": np.ascontiguousarray(shards[c])} for c in range(NCORES)]
    res = run_bass_kernel_spmd(nc, in_maps, list(range(NCORES)), trace=trace)
    mask = np.concatenate([r["mask"] for r in res.results], axis=0)
    flags = np.concatenate([r["flags"] for r in res.results], axis=0).reshape(-1)
    return mask, flags, res


def kernel(**inputs) -> np.ndarray:
    starts = np.ascontiguousarray(np.asarray(inputs["starts"]), dtype=np.int32)
    t_in = int(np.asarray(inputs["T"]))
    l_in = int(np.asarray(inputs["l"]))
    assert starts.shape == (B, NSEG), starts.shape
    assert t_in == T and l_in == L, (t_in, l_in)

    mask_u8, flags, _ = run_device(starts)
    mask = mask_u8.astype(bool)

    bad_rows = np.nonzero(flags != 0)[0]
    for r in bad_rows:  # pathological rows: exact host recompute (never on real data)
        mask[r] = _host_exact_row(starts[r])
    return mask
